# revision 2
# baseline (speedup 1.0000x reference)
"""Causal self-attention Trainium2 kernel v2 (B=4, T=2048, C=2048, H=16).

Sharding: 8 cores = 4 batches x 2 head-groups (8 heads each).

Per head, attention runs in S^T orientation (FlashAttention-style
unnormalized accumulation):
  S^T_j = K_j @ Q^T          (lhsT=kT block j, rhs=qT cols >= j*128)
  A'^T  = exp(S^T)           (no max subtraction: |S| <= ~2.5)
  O'_i  = sum_j A'_{i,j} [V_j | 1]   (ones column => col 128 = softmax denom l)
  O_i   = O'_i * (1/l)       (per-partition scale at PSUM evac)
  oT_i  = O_i^T              (PE transpose, 16/head)
Projection: partial_p[t, :] = sum_{h in pair p} O_h @ Wproj_h, four
head-pair partials per core, bf16, summed on host with b_proj.
"""

import math
import numpy as np
import ml_dtypes
from contextlib import ExitStack

import concourse.bass as bass
import concourse.tile as tile
from concourse import bacc, mybir
from concourse import bass_utils

BF16 = mybir.dt.bfloat16
F32 = mybir.dt.float32
AF = mybir.ActivationFunctionType

B, T, C, H = 4, 2048, 2048, 16
D = 128          # head dim
NH = 8           # heads per core
NCC = 16         # contraction chunks of 128 over C
NTT = 16         # t tiles of 128
TCH = 1024       # psum free-dim chunk (2 banks)
LAG = 2          # steps between S^T_i and AV_i


def build_program():
    nc = bacc.Bacc(
        "TRN2",
        target_bir_lowering=False,
        debug=False,
        enable_asserts=False,
        num_devices=8,
    )

    xT = nc.dram_tensor("xT", [128, NCC, T], BF16, kind="ExternalInput").ap()
    wqkv = nc.dram_tensor("wqkv", [NH, 3, 128, NCC, 128], BF16, kind="ExternalInput").ap()
    wproj = nc.dram_tensor("wproj", [NH, 128, C], BF16, kind="ExternalInput").ap()
    biasd = nc.dram_tensor("biasd", [128, 32], F32, kind="ExternalInput").ap()
    maskd = nc.dram_tensor("maskd", [128, 128], BF16, kind="ExternalInput").ap()
    identd = nc.dram_tensor("identd", [128, 128], BF16, kind="ExternalInput").ap()
    partials = [
        nc.dram_tensor(f"part{p}", [NTT, 128, C], BF16, kind="ExternalOutput").ap()
        for p in range(4)
    ]
    # head-7-only contributions for the last 4 t-tiles (true kernel tail)
    part4 = nc.dram_tensor("part4", [4, 128, C], BF16, kind="ExternalOutput").ap()

    with tile.TileContext(nc) as tc, ExitStack() as ctx:
        const_pool = ctx.enter_context(tc.tile_pool(name="const", bufs=1))
        xt_pool = ctx.enter_context(tc.tile_pool(name="xt", bufs=1))
        wt_pool = ctx.enter_context(tc.tile_pool(name="wt", bufs=3))
        qk_pool = ctx.enter_context(tc.tile_pool(name="qk", bufs=2))
        vt_pool = ctx.enter_context(tc.tile_pool(name="vt", bufs=2))
        va_pool = ctx.enter_context(tc.tile_pool(name="va", bufs=2))
        a_pool = ctx.enter_context(tc.tile_pool(name="a", bufs=1))
        os_pool = ctx.enter_context(tc.tile_pool(name="os", bufs=3))
        ot_pool = ctx.enter_context(tc.tile_pool(name="ot", bufs=1))
        wp_pool = ctx.enter_context(tc.tile_pool(name="wp", bufs=4))
        st_pool = ctx.enter_context(tc.tile_pool(name="st", bufs=4))
        ev_pool = ctx.enter_context(tc.tile_pool(name="ev", bufs=4))
        ps_big = ctx.enter_context(tc.tile_pool(name="psb", bufs=3, space="PSUM"))
        ps_av = ctx.enter_context(tc.tile_pool(name="psa", bufs=1, space="PSUM"))
        ps_tr = ctx.enter_context(tc.tile_pool(name="pstr", bufs=1, space="PSUM"))

        bias_sb = const_pool.tile([128, 32], F32, tag="bias")
        nc.sync.dma_start(bias_sb[:], biasd[:])
        mask_sb = const_pool.tile([128, 128], BF16, tag="mask")
        nc.sync.dma_start(mask_sb[:], maskd[:])
        ident_sb = const_pool.tile([128, 128], BF16, tag="ident")
        nc.sync.dma_start(ident_sb[:], identd[:])
        xt_sb = xt_pool.tile([128, NCC, T], BF16, tag="xt")
        for cc in range(NCC):
            eng = (nc.sync, nc.gpsimd, nc.scalar)[cc % 3]
            eng.dma_start(xt_sb[:, cc, :], xT[:, cc, :])

        qkt = {}    # h -> (qT, kT)
        vaug = {}   # h -> v_aug tile [128, NTT, 129]
        at = {}     # j -> a_t tile (current head)
        oT = {}     # h -> oT tile [128, T]
        wp_tiles = {}

        # single-bank manual rings: 2x132 f32 AV slots, 4x128 bf16 transpose
        # slots (separate accumulation groups within one PSUM bank)
        po_bank = ps_av.tile([128, 396], F32, tag="po")
        pt_bank = ps_tr.tile([128, 4, 128], BF16, tag="pt")
        slot_ctr = {"po": 0, "pt": 0}

        def po_slot():
            r = slot_ctr["po"] % 3
            slot_ctr["po"] += 1
            return po_bank[:, r * 132:r * 132 + 132]

        def pt_slot():
            r = slot_ctr["pt"] % 4
            slot_ctr["pt"] += 1
            return pt_bank[:, r, :]

        # ---- QKV units for head h (woven into span h-1) ----
        def qkv_units(h):
            units = []
            wts = {}

            def load_w(mat):
                wt = wt_pool.tile([128, NCC, 128], BF16, tag="wt")
                nc.gpsimd.dma_start(wt[:], wqkv[h, mat])
                wts[mat] = wt

            qT = qk_pool.tile([128, T], BF16, tag="qT")
            kT = qk_pool.tile([128, T], BF16, tag="kT")
            vT = vt_pool.tile([128, T], BF16, tag="vT")
            qkt[h] = (qT, kT)

            def mm_group(mat, dst, tch):
                def emit():
                    if tch == 0:
                        load_w(mat)
                    wt = wts[mat]
                    ps = ps_big.tile([128, TCH], F32, tag="pb")
                    for s0 in range(0, TCH, 512):
                        for cc in range(NCC):
                            nc.tensor.matmul(
                                ps[:, s0:s0 + 512],
                                lhsT=wt[:, cc, :],
                                rhs=xt_sb[:, cc, tch * TCH + s0:tch * TCH + s0 + 512],
                                start=(cc == 0),
                                stop=(cc == NCC - 1),
                            )
                    bias_ap = bias_sb[:, mat * 8 + h:mat * 8 + h + 1]
                    if mat == 2:
                        nc.vector.tensor_scalar_add(
                            dst[:, tch * TCH:(tch + 1) * TCH], ps[:], bias_ap
                        )
                    else:
                        nc.scalar.activation(
                            dst[:, tch * TCH:(tch + 1) * TCH], ps[:],
                            AF.Identity, bias=bias_ap,
                        )
                return emit

            for mat, dst in ((0, qT), (1, kT), (2, vT)):
                for tch in range(T // TCH):
                    units.append(mm_group(mat, dst, tch))

            def vtrans():
                v_aug = va_pool.tile([128, NTT, 129], BF16, tag="va")
                vaug[h] = v_aug
                nc.vector.memset(v_aug[:, :, 128:129], 1.0)
                for jj in range(NTT):
                    pt = pt_slot()
                    nc.tensor.matmul(
                        pt, lhsT=vT[:, jj * 128:(jj + 1) * 128], rhs=ident_sb[:],
                        is_transpose=True, skip_group_check=True,
                    )
                    if jj % 2 == 0:
                        nc.vector.tensor_copy(v_aug[:, jj, 0:128], pt)
                    else:
                        nc.scalar.copy(v_aug[:, jj, 0:128], pt)
            units.append(vtrans)
            return units

        # ---- attention steps for head h ----
        def st_row(h, j):
            """S^T_j = K_j @ Q^T for q cols >= j*128, masked diag, exp."""
            qT, kT = qkt[h]
            ncols = (NTT - j) * 128
            a_t = a_pool.tile([128, ncols], BF16, tag=f"a{j}")
            at[j] = a_t
            nch = (ncols + TCH - 1) // TCH
            for c in range(nch):
                c0 = c * TCH
                cw = min(TCH, ncols - c0)
                ps = ps_big.tile([128, TCH], F32, tag="pb")
                for s0 in range(0, cw, 512):
                    sw = min(512, cw - s0)
                    nc.tensor.matmul(
                        ps[:, s0:s0 + sw],
                        lhsT=kT[:, j * 128:(j + 1) * 128],
                        rhs=qT[:, j * 128 + c0 + s0:j * 128 + c0 + s0 + sw],
                        start=True, stop=True,
                    )
                nc.scalar.activation(a_t[:, c0:c0 + cw], ps[:, :cw], AF.Exp)
                if c == 0:
                    # causal zeroing of the diagonal tile, off the PSUM path
                    nc.vector.tensor_mul(a_t[:, 0:128], a_t[:, 0:128], mask_sb[:])

        def av_row(h, i):
            """O'_i = sum_{j<=i} A'^T_{j,i}^T @ [V_j | 1]; normalize; stage O_i."""
            v_aug = vaug[h]
            po = po_slot()
            for j in range(i + 1):
                nc.tensor.matmul(
                    po[:, 0:129],
                    lhsT=at[j][:, (i - j) * 128:(i - j + 1) * 128],
                    rhs=v_aug[:, j, :],
                    start=(j == 0),
                    stop=(j == i),
                    skip_group_check=True,
                )
            linv = st_pool.tile([128, 1], F32, tag="linv")
            nc.vector.reciprocal(linv[:], po[:, 128:129])
            o_sb = os_pool.tile([128, 128], BF16, tag="os")
            nc.scalar.activation(o_sb[:], po[:, 0:128], AF.Copy, scale=linv[:])
            return o_sb

        def otrans(h, i, o_sb):
            pt = pt_slot()
            nc.tensor.matmul(
                pt, lhsT=o_sb[:], rhs=ident_sb[:],
                is_transpose=True, skip_group_check=True,
            )
            nc.vector.tensor_copy(oT[h][:, i * 128:(i + 1) * 128], pt)

        # ---- projection units (head pair p, t tile tt) ----
        def load_wp(p):
            for h in (2 * p, 2 * p + 1):
                wp_t = wp_pool.tile([128, C], BF16, tag="wp", name=f"wp{h}")
                nc.sync.dma_start(wp_t[:], wproj[h])
                wp_tiles[h] = wp_t

        dma_rr = {"i": 0}

        def out_dma(dst, src):
            eng = (nc.gpsimd, nc.sync, nc.scalar)[dma_rr["i"] % 3]
            dma_rr["i"] += 1
            eng.dma_start(dst, src)

        def proj_unit(p, tt, heads=None, dstt=None):
            def emit():
                hs = heads if heads is not None else (2 * p, 2 * p + 1)
                for cq in range(C // TCH):
                    ps = ps_big.tile([128, TCH], F32, tag="pb")
                    for s0 in range(0, TCH, 512):
                        for ki, hh in enumerate(hs):
                            nc.tensor.matmul(
                                ps[:, s0:s0 + 512],
                                lhsT=oT[hh][:, tt * 128:(tt + 1) * 128],
                                rhs=wp_tiles[hh][:, cq * TCH + s0:cq * TCH + s0 + 512],
                                start=(ki == 0),
                                stop=(ki == len(hs) - 1),
                            )
                    ev = ev_pool.tile([128, TCH], BF16, tag="ev")
                    if cq % 2 == 0:
                        nc.vector.tensor_copy(ev[:], ps[:])
                    else:
                        nc.scalar.copy(ev[:], ps[:])
                    dst_t = dstt if dstt is not None else partials[p][tt]
                    out_dma(dst_t[:, cq * TCH:(cq + 1) * TCH], ev[:])
            return emit

        # ---- schedule ----
        # Pairs 0-2 weave as whole units into spans 2..7 once fully ready;
        # pair 3 weaves into span 7 per-tt (as oT[7] tiles land) with the
        # remainder after the last span.
        proj_q = [(p, tt) for p in range(3) for tt in range(NTT)]
        pulls = {0: 0, 1: 0, 2: 5, 3: 5, 4: 6, 5: 6, 6: 10, 7: 16}
        pulled = 0

        for u in qkv_units(0):
            u()

        for h in range(NH):
            ua = list(qkv_units(h + 1)) if h + 1 < NH else []
            if h % 2 == 1 and (h - 1) // 2 < 3:
                load_wp((h - 1) // 2)
            for _ in range(pulls[h]):
                if pulled < len(proj_q) and proj_q[pulled][0] * 2 + 1 <= h - 1:
                    ua.append(proj_unit(*proj_q[pulled]))
                    pulled += 1
            if h == 7:
                load_wp(3)
                # pair3 full units for tt<12 as oT[7] tiles land; h6-only
                # units for tt>=12 are unconstrained fillers
                early = [(tt, proj_unit(3, tt)) for tt in range(12)]
                ua += [proj_unit(3, tt, heads=(6,)) for tt in range(12, NTT)]
            else:
                early = []
            oT[h] = ot_pool.tile([128, T], BF16, tag=f"oT{h}", name=f"oT{h}")
            o_stage = {}
            ui = 0
            ei = 0
            for i in range(NTT):
                st_row(h, i)
                if i >= LAG:
                    o_stage[i - LAG] = av_row(h, i - LAG)
                if i >= LAG + 1:
                    otrans(h, i - LAG - 1, o_stage.pop(i - LAG - 1))
                if ei < len(early) and early[ei][0] <= i - LAG - 2:
                    early[ei][1]()
                    ei += 1
                if ui < len(ua):
                    ua[ui]()
                    ui += 1
                if ui < len(ua) and i % 2 == 1:
                    ua[ui]()
                    ui += 1
            for i in range(NTT - LAG, NTT):
                o_stage[i] = av_row(h, i)
            for i in range(NTT - LAG - 1, NTT):
                otrans(h, i, o_stage.pop(i))
            while ui < len(ua):
                ua[ui]()
                ui += 1
            while ei < len(early):
                early[ei][1]()
                ei += 1

        for tt in range(12, NTT):
            proj_unit(3, tt, heads=(7,), dstt=part4[tt - 12])()

    nc.compile()
    return nc


_NC = None


def _get_nc():
    global _NC
    if _NC is None:
        _NC = build_program()
    return _NC


def make_in_maps(x, w_qkv, b_qkv, w_proj, b_proj):
    bf = ml_dtypes.bfloat16
    s = 1.0 / math.sqrt(D)
    # S^T tile [k(part), q(free)]: keep where k <= q, zero otherwise
    mask = np.where(
        np.arange(128)[None, :] >= np.arange(128)[:, None], 1.0, 0.0
    ).astype(bf)

    xTs = []
    for b in range(B):
        xt = np.ascontiguousarray(x[b].T).reshape(NCC, 128, T).transpose(1, 0, 2)
        xTs.append(np.ascontiguousarray(xt).astype(bf))

    in_maps = []
    for core in range(8):
        b, g = core // 2, core % 2
        wq_arr = np.empty((NH, 3, 128, NCC, 128), np.float32)
        bias = np.zeros((128, 32), np.float32)
        wp_arr = np.empty((NH, 128, C), np.float32)
        for hi in range(NH):
            hgl = g * NH + hi
            wq = w_qkv[hgl * D:(hgl + 1) * D, :] * s          # [D, C]
            wk = w_qkv[C + hgl * D:C + (hgl + 1) * D, :]
            wv = w_qkv[2 * C + hgl * D:2 * C + (hgl + 1) * D, :]
            for mat, wm in ((0, wq), (1, wk), (2, wv)):
                wq_arr[hi, mat] = wm.T.reshape(NCC, 128, 128).transpose(1, 0, 2)
            bias[:, hi] = b_qkv[hgl * D:(hgl + 1) * D] * s
            bias[:, 8 + hi] = b_qkv[C + hgl * D:C + (hgl + 1) * D]
            bias[:, 16 + hi] = b_qkv[2 * C + hgl * D:2 * C + (hgl + 1) * D]
            wp_arr[hi] = w_proj[:, hgl * D:(hgl + 1) * D].T   # [128(d), C]
        in_maps.append({
            "xT": xTs[b],
            "wqkv": wq_arr.astype(bf),
            "wproj": wp_arr.astype(bf),
            "biasd": bias,
            "maskd": mask,
            "identd": np.eye(128, dtype=np.float32).astype(bf),
        })
    return in_maps


def run_cores(in_maps, trace=False, **kw):
    nc = _get_nc()
    if trace:
        import sys, types
        if "antenv.axon_hooks" not in sys.modules:
            from trn_agent_boot.trn_boot import _ntff_profile_via_ctypes
            hook = _ntff_profile_via_ctypes("/opt/axon/libaxon_pjrt.so")
            mod = types.ModuleType("antenv.axon_hooks")
            mod.get_axon_ntff_profile_hook = lambda: hook
            sys.modules["antenv.axon_hooks"] = mod
    return bass_utils.run_bass_kernel_spmd(
        nc, in_maps, core_ids=list(range(8)), trace=trace, **kw
    )


def kernel(x, w_qkv, b_qkv, w_proj, b_proj):
    x = np.asarray(x, np.float32)
    w_qkv = np.asarray(w_qkv, np.float32)
    b_qkv = np.asarray(b_qkv, np.float32)
    w_proj = np.asarray(w_proj, np.float32)
    b_proj = np.asarray(b_proj, np.float32)

    in_maps = make_in_maps(x, w_qkv, b_qkv, w_proj, b_proj)
    res = run_cores(in_maps, trace=False)
    out = np.empty((B, T, C), np.float32)
    for b in range(B):
        acc = b_proj.astype(np.float32)[None, :] + np.zeros((T, C), np.float32)
        for c in (2 * b, 2 * b + 1):
            for p in range(4):
                acc += res.results[c][f"part{p}"].astype(np.float32).reshape(T, C)
            acc[12 * 128:] += res.results[c]["part4"].astype(np.float32).reshape(512, C)
        out[b] = acc
    return out
